# revision 47
# baseline (speedup 1.0000x reference)
"""Bilinear pooling kernel for 8 Trainium2 NeuronCores (Bass/Tile).

Math (matches the jax reference):
  x = concat([x1, x2, x3], channel) -> (B=64, M=147, L=3136)
  phi_b = x_b @ x_b.T                              (147, 147), symmetric
  phi = sign(phi) * sqrt(|phi| + EPS)              (signed sqrt)
  phi = phi / sqrt(sum(phi^2 + EPS) + 1.0)         (per-batch normalize)
  h = phi_vec @ fc0_w.T + fc0_b                    (64, 1024)
  y = h @ fc1_w.T + fc1_b                          (64, 64)
  logits = y @ fc2_w.T + fc2_b                     (64, 4)
  merged = softmax(concat([logits, x11, x21, x31]))
  x_merge = merged @ cls_w.T + cls_b               (64, 4)

Distribution (v2):
  - phase 1 is batch-parallel (8 batches/core).  Because phi is symmetric,
    only rows 0:128 (A block, 128x147) and the 19x19 diagonal block are
    computed; the mirror block is recovered by folding fc0's weights
    host-side.  The 19x19 block is built from 5 stacked matmuls (5 l-chunks
    side by side in the PE array) instead of 25 thin ones.
  - normalization is deferred: phase 1 ships UNNORMALIZED signed-sqrt phi
    (fp16) plus per-batch |phi| totals; the 1/sqrt(total+C) scale is applied
    to the 4-wide logits after the final AllReduce (everything in between is
    linear in phi).
  - fc0 is contraction(i)-sharded: an AllToAll gives each core a 2432-wide
    i-slice of every batch's phi vector (~150 KB per collective vs 2.8 MB
    for the old AllGather).  Each core computes partial h (1024) -> partial
    y (64) -> partial z = W2 y (4x64); one tiny AllReduce of (4,64)+totals
    finishes the linear chain, then the softmax tail runs replicated.
  - all DRAM inputs are laid out host-side in exact SBUF layout so every
    DMA moves contiguous multi-KB partition lines.
"""

import sys

sys.path.insert(0, "/opt/trn_rl_repo")

import numpy as np

import concourse.bass as bass
import concourse.tile as tile
from concourse import masks, mybir
from concourse.bass_utils import run_bass_kernel_spmd
import bass_rust
from bass_rust import ScopedClock

# ---------------------------------------------------------------------------
# Workaround: this toolchain's walrus accepts only ONE semaphore wait per
# instruction, but Tile can attach several.  Split excess waits onto
# same-engine nops placed immediately before the instruction (same engine
# => executed in order, so synchronization semantics are unchanged).
# ---------------------------------------------------------------------------
_MAX_WAITS = 1
_ws_counter = [0]


def _split_excess_waits(obb):
    for bb, insts in list(obb.items()):
        new_list = []
        for inst in insts:
            info = inst.sync_info
            if info is not None and len(info.on_wait) > _MAX_WAITS:
                waits = list(info.on_wait)
                excess = waits[:-_MAX_WAITS]
                keep = waits[-_MAX_WAITS:]
                for i in range(0, len(excess), _MAX_WAITS):
                    _ws_counter[0] += 1
                    nop = mybir.InstNoOp(
                        name=f"WS-{_ws_counter[0]}",
                        sync_info=bass_rust.SyncInfo(
                            on_wait=excess[i : i + _MAX_WAITS], on_update=[]
                        ),
                        bass_nofuse=True,
                        engine=inst.engine,
                    )
                    new_list.append(nop)
                inst.sync_info = bass_rust.SyncInfo(
                    on_wait=keep, on_update=list(info.on_update)
                )
            new_list.append(inst)
        obb[bb] = new_list


_RealTCW = tile.TileClockWait


class _TCWWrapper:
    def __init__(self, *args, **kwargs):
        self._inner = _RealTCW(*args, **kwargs)
        self._obb = (
            args[1] if len(args) > 1 else kwargs["ordered_instructions_by_block"]
        )

    def __getattr__(self, name):
        return getattr(self._inner, name)

    def assign_waits(self, bb_name):
        self._inner.assign_waits(bb_name)
        _split_excess_waits(self._obb)


tile.TileClockWait = _TCWWrapper


def _split_drain_and_barrier(self, tick_clock, wait_clock):
    nc = self.nc
    drain_inst = nc.sync.drain()
    wait_clock.add_sem_waits(
        drain_inst.ins, ScopedClock({None: tick_clock.global_clock})
    )
    info = drain_inst.ins.sync_info
    if info is not None and len(info.on_wait) > _MAX_WAITS:
        waits = list(info.on_wait)
        drain_inst.ins.sync_info = bass_rust.SyncInfo(
            on_wait=waits[:_MAX_WAITS], on_update=list(info.on_update)
        )
        rest = waits[_MAX_WAITS:]
        while rest:
            chunk, rest = rest[:_MAX_WAITS], rest[_MAX_WAITS:]
            nop_inst = nc.sync.nop(nofuse=True, hint="tail_drain_split")
            nop_inst.ins.sync_info = bass_rust.SyncInfo(on_wait=chunk, on_update=[])
    nc.all_engine_barrier()
    assert self.sems is not None
    popped = nc._tile_sem_poison_stack.pop()
    assert popped is self._sem_poison
    nc.clear_and_free_semaphores(list(self.sems.allocated().values()))
    nc.all_engine_barrier()


tile.TileContext._drain_and_barrier = _split_drain_and_barrier

# ---------------------------------------------------------------------------
# Problem constants (hardcoded per the spec)
# ---------------------------------------------------------------------------
N_CORES = 8
CORE_IDS = list(range(N_CORES))
B = 64
B_LOC = B // N_CORES  # 8 batches per core
C = 49
L = 3136  # 56*56
LCH = 25  # l-chunks of 128 (last one zero-padded: 3136 = 24*128 + 64)
M = 147  # 3*49 channels
O0 = 1024  # fc0 out features
HID = 64
CLS = 4
EPS = 1e-8
MM = M * M
# normalizer: sum(phi_ss^2 + EPS) + 1.0 == sum|phi| + 2*MM*EPS + 1.0
NORM_C = float(2 * MM * EPS + 1.0)
TOT_SCALE = 256.0  # |phi| totals are carried as fp16 scaled by 1/256

RA = 16  # A-block rows per destination core
RB = 3  # padded-B rows per destination core (B 19x19 padded to 24 rows)
SLICE = 2432  # = 19*128 per-batch per-dest phi slice (2352 A + 57 B + 23 pad)
KCH = SLICE // 128  # 19 fc0 contraction chunks
NB = 8  # fc0 output blocks of 128

F32 = mybir.dt.float32
F16 = mybir.dt.float16


def _build_nc():
    nc = bass.Bass()

    # -- external I/O ------------------------------------------------------
    # all big tensors arrive in exact SBUF layout (partition-major).
    xall_d = nc.dram_tensor("xall", [B_LOC, 128, LCH * M], F16, kind="ExternalInput")
    w0_d = nc.dram_tensor("w0f", [128, KCH * O0], F16, kind="ExternalInput")
    w1_d = nc.dram_tensor("w1t", [128, NB * HID], F32, kind="ExternalInput")
    w2_d = nc.dram_tensor("w2t", [HID, CLS], F32, kind="ExternalInput")
    wct_d = nc.dram_tensor("wct", [4 * CLS + 1, CLS], F32, kind="ExternalInput")
    kb_d = nc.dram_tensor("kb", [B, CLS], F32, kind="ExternalInput")
    xm_d = nc.dram_tensor("xm", [B, 3 * CLS], F32, kind="ExternalInput")
    logits_d = nc.dram_tensor("logits", [B, CLS], F32, kind="ExternalOutput")
    xmerge_d = nc.dram_tensor("x_merge", [B, CLS], F32, kind="ExternalOutput")

    IN_H = B_LOC * SLICE  # 19456 elems per dest row
    # the 8 per-batch |phi| totals ride inside the 23-elem pad of the last
    # batch slice (W rows there are zero, so fc0 ignores them)
    TOT_OFF = 7 * SLICE + RA * M + RB * 19

    with tile.TileContext(nc) as tc:
        with tc.tile_pool(name="dram", bufs=1, space="DRAM") as dram, tc.tile_pool(
            name="const", bufs=1
        ) as const:
            a2a_in = dram.tile([N_CORES, IN_H], F16)
            recv = dram.tile([N_CORES, IN_H], F16)
            ar_in = dram.tile([CLS, B], F32)
            ar_out = dram.tile([CLS, B], F32, addr_space="Shared")

            # -- constants ----------------------------------------------
            identf = const.tile([128, 128], F32)
            masks.make_identity(nc, identf[:])
            ones16 = const.tile([128, 8], F16)
            nc.gpsimd.memset(ones16[:], 1.0)
            eps_col = const.tile([128, 1], F32)
            nc.gpsimd.memset(eps_col[:], EPS)
            normc_col = const.tile([B, 1], F32)
            nc.gpsimd.memset(normc_col[:], NORM_C)
            zpad = const.tile([64, SLICE - RA * M - RB * 19], F16)
            nc.gpsimd.memset(zpad[:], 0.0)

            # ===========================================================
            # phase 0: input loads.  x batches first (they gate phase 1),
            # then the fc0 weight slab (only needed much later).  Split
            # across BOTH hwdge engines: descriptor generation and ring
            # drain interlock per engine, so two pipelines double the
            # effective issue rate.  Small/const loads go to gpsimd.
            # ===========================================================
            xt = const.tile([128, B_LOC, LCH, M], F16)
            for b in range(B_LOC):
                eng = nc.sync if b % 2 == 0 else nc.gpsimd
                eng.dma_start(
                    xt[:, b], xall_d[b].rearrange("p (lc m) -> p lc m", lc=LCH)
                )
            w_sb = const.tile([128, KCH, O0], F16)
            for kg in range(4):
                k0, k1 = 5 * kg, min(5 * (kg + 1), KCH)
                nc.sync.dma_start(
                    w_sb[:, k0:k1],
                    w0_d[:, k0 * O0 : k1 * O0].rearrange("p (k o) -> p k o", o=O0),
                )
            w1_sb = const.tile([128, NB, HID], F32)
            nc.scalar.dma_start(
                w1_sb[:], w1_d.rearrange("p (n h) -> p n h", h=HID)
            )
            w2_sb = const.tile([HID, CLS], F32)
            nc.scalar.dma_start(w2_sb[:], w2_d[:])
            wc_sb = const.tile([4 * CLS + 1, CLS], F32)
            nc.scalar.dma_start(wc_sb[:], wct_d[:])
            kb_sb = const.tile([B, CLS], F32)
            nc.scalar.dma_start(kb_sb[:], kb_d[:])
            xm_sb = const.tile([B, 3 * CLS], F32)
            nc.scalar.dma_start(xm_sb[:], xm_d[:])
            merged = const.tile([B, 4 * CLS], F32)
            nc.vector.tensor_copy(merged[:, CLS:], xm_sb[:])
            smx = const.tile([B, 4 * CLS + 1], F32)
            nc.vector.memset(smx[:, 4 * CLS :], 1.0)
            # zero the 23-elem pad of every (dest, batch) slice once
            nc.scalar.dma_start(
                a2a_in.rearrange("d (b i) -> (d b) i", b=B_LOC)[
                    :, RA * M + RB * 19 :
                ],
                zpad[:],
            )
            # phiT lives here so the first transpose can issue mid-phase-1
            phiT = const.tile([128, KCH, B], F16)

            # per-batch |phi| row sums, accumulated across phase 1
            rsum = const.tile([128, B_LOC, 2], F32)
            rsumB = const.tile([M - 128, B_LOC], F32)
            # padded signed-sqrt B block; rows 19:24 zeroed once via DMA
            # (engine APs must start at a 32-aligned partition)
            nBp = const.tile([8 * RB, 19], F16)
            nc.gpsimd.dma_start(nBp[19 : 8 * RB, :], zpad[0 : 8 * RB - 19, 0:19])
            # B columns repacked at 32-col pitch so the per-chunk diagonal
            # blocks land on 32-aligned partitions; cols 19:32 stay zero
            bpack = const.tile([128, LCH, 32], F16)
            nc.vector.memset(bpack[:], 0.0)

            # ===========================================================
            # phase 1: bilinear + signed sqrt, per batch (normalization
            # deferred to the tail).  PE stream has NO mid-stream stalls.
            # ===========================================================
            with tc.tile_pool(name="p1sb", bufs=3) as sb, tc.tile_pool(
                name="p1psA", bufs=3, space="PSUM"
            ) as psA, tc.tile_pool(
                name="p1psB", bufs=2, space="PSUM"
            ) as psB, nc.named_scope("p1_bilinear"):
                for b in range(B_LOC):
                    bl = b

                    # A block: phi rows 0:128 x cols 0:147
                    pA = psA.tile([128, M], F32, tag="pA")
                    for lc in range(LCH):
                        nc.tensor.matmul(
                            pA[:],
                            xt[:, b, lc, 0:128],
                            xt[:, b, lc, :],
                            start=(lc == 0),
                            stop=(lc == LCH - 1),
                        )
                    # B diagonal 19x19: 4 l-chunks stacked side by side at
                    # 32-col pitch; diag 32-blocks of the 128x128 result
                    # hold the per-chunk contributions, summed below on DVE.
                    nc.vector.tensor_copy(bpack[:, :, 0:19], xt[:, b, :, 128:M])
                    pB5 = psB.tile([128, 128], F32, tag="pB5")
                    for s in range(6):
                        ap = bpack[:, 4 * s : 4 * s + 4, :]
                        nc.tensor.matmul(
                            pB5[:], ap, ap, start=(s == 0), stop=False
                        )
                    ap = bpack[:, 24, :]
                    nc.tensor.matmul(
                        pB5[0:32, 0:32], ap, ap, start=False, stop=True
                    )

                    # norm chain (ACT + DVE only; PE keeps streaming)
                    sgnA = sb.tile([128, M], F32, tag="sgnA")
                    absA = sb.tile([128, M], F32, tag="absA")
                    nc.scalar.activation(
                        sgnA[:], pA[:], mybir.ActivationFunctionType.Sign
                    )
                    nc.scalar.activation(
                        absA[:], pA[:], mybir.ActivationFunctionType.Abs
                    )
                    nc.vector.reduce_sum(
                        rsum[:, b, 0:1], absA[:], axis=mybir.AxisListType.X
                    )
                    # mirror columns 128:147 count twice in the full |phi| sum
                    nc.vector.reduce_sum(
                        rsum[:, b, 1:2], absA[:, 128:M], axis=mybir.AxisListType.X
                    )
                    sqA = sb.tile([128, M], F32, tag="sqA")
                    nc.scalar.activation(
                        sqA[:],
                        absA[:],
                        mybir.ActivationFunctionType.Sqrt,
                        bias=eps_col[:],
                    )
                    nA = sb.tile([128, M], F16, tag="nA")
                    nc.vector.tensor_mul(nA[:], sqA[:], sgnA[:])
                    # early phi writes ride gpsimd (its queue is free; the
                    # collective instruction would block anything behind it);
                    # later ones go to scalar, whose small loads have drained
                    wr_eng = nc.gpsimd if b < 4 else nc.scalar
                    wr_eng.dma_start(
                        a2a_in[:, bl * SLICE : bl * SLICE + RA * M].rearrange(
                            "d (r m) -> d r m", r=RA
                        ),
                        nA[:],
                    )

                    bsum = sb.tile([19, 19], F32, tag="bsum")
                    nc.vector.tensor_copy(bsum[:], pB5[0:19, 0:19])
                    nc.vector.tensor_add(bsum[:], bsum[:], pB5[32:51, 32:51])
                    nc.vector.tensor_add(bsum[:], bsum[:], pB5[64:83, 64:83])
                    nc.vector.tensor_add(bsum[:], bsum[:], pB5[96:115, 96:115])
                    sgnB = sb.tile([19, 19], F32, tag="sgnB")
                    absB = sb.tile([19, 19], F32, tag="absB")
                    nc.scalar.activation(
                        sgnB[:], bsum[:], mybir.ActivationFunctionType.Sign
                    )
                    nc.scalar.activation(
                        absB[:], bsum[:], mybir.ActivationFunctionType.Abs
                    )
                    nc.vector.reduce_sum(
                        rsumB[:, b : b + 1], absB[:], axis=mybir.AxisListType.X
                    )
                    sqB = sb.tile([19, 19], F32, tag="sqB")
                    nc.scalar.activation(
                        sqB[:],
                        absB[:],
                        mybir.ActivationFunctionType.Sqrt,
                        bias=eps_col[0:19],
                    )
                    nc.vector.tensor_mul(nBp[0:19, :], sqB[:], sgnB[:])
                    wr_eng.dma_start(
                        a2a_in[
                            :, bl * SLICE + RA * M : bl * SLICE + RA * M + RB * 19
                        ].rearrange("d (s c) -> d s c", s=RB),
                        nBp[:],
                    )

                # per-batch |phi| totals: cross-partition sum via one
                # fp16 ones-matmul (PE is idle by now), scaled by 1/256
                # to fit fp16 on the wire.
                rs_sum = sb.tile([128, B_LOC], F32, tag="rs_sum")
                nc.vector.tensor_add(rs_sum[:], rsum[:, :, 0], rsum[:, :, 1])
                rs16 = sb.tile([128, B_LOC], F16, tag="rs16")
                nc.vector.tensor_scalar_mul(rs16[:], rs_sum[:], 1.0 / TOT_SCALE)
                rsB16 = sb.tile([M - 128, B_LOC], F16, tag="rsB16")
                nc.vector.tensor_scalar_mul(rsB16[:], rsumB[:], 1.0 / TOT_SCALE)
                tot_ps = psB.tile([8, 8], F32, tag="tot")
                nc.tensor.matmul(
                    tot_ps[:], ones16[:, :], rs16[:], start=True, stop=False
                )
                nc.tensor.matmul(
                    tot_ps[:],
                    ones16[0 : M - 128, :],
                    rsB16[:],
                    start=False,
                    stop=True,
                )
                tot16 = sb.tile([8, 8], F16, tag="tot16")
                nc.scalar.copy(tot16[:], tot_ps[:])
                nc.scalar.dma_start(
                    a2a_in[:, TOT_OFF : TOT_OFF + B_LOC], tot16[:]
                )

            with nc.named_scope("p2_a2a"):
                nc.gpsimd.collective_compute(
                    "AllToAll",
                    mybir.AluOpType.bypass,
                    replica_groups=[CORE_IDS],
                    ins=[a2a_in.opt()],
                    outs=[recv.opt()],
                )

            # ===========================================================
            # phase 3: transpose received phi slices, fc0/fc1/fc2 partials
            # column j of phiT holds batch j (natural order)
            # ===========================================================
            with tc.tile_pool(name="p3sb", bufs=1) as sb3, tc.tile_pool(
                name="p3ps", bufs=1, space="PSUM"
            ) as ps3, nc.named_scope("p3_fc0"):
                nc.sync.dma_start_transpose(
                    phiT[:],
                    recv.rearrange("d (b i) -> (d b) i", b=B_LOC),
                )

                h_sb = sb3.tile([128, NB, B], F32)
                for ob in range(NB):
                    ph = ps3.tile([128, B], F32, tag=f"h{ob % 2}", bufs=2)
                    for k in range(KCH):
                        nc.tensor.matmul(
                            ph[:],
                            w_sb[:, k, 128 * ob : 128 * (ob + 1)],
                            phiT[:, k, :],
                            start=(k == 0),
                            stop=(k == KCH - 1),
                        )
                    nc.scalar.copy(h_sb[:, ob, :], ph[:])

                py = ps3.tile([HID, B], F32, tag="py")
                for ob in range(NB):
                    nc.tensor.matmul(
                        py[:],
                        w1_sb[:, ob, :],
                        h_sb[:, ob, :],
                        start=(ob == 0),
                        stop=(ob == NB - 1),
                    )
                y_sb = sb3.tile([HID, B], F32)
                nc.vector.tensor_copy(y_sb[:], py[:])
                pz = ps3.tile([CLS, B], F32, tag="pz")
                nc.tensor.matmul(pz[:], w2_sb[:], y_sb[:], start=True, stop=True)
                z_sb = sb3.tile([CLS, B], F32)
                nc.scalar.copy(z_sb[:], pz[:])
                nc.scalar.dma_start(ar_in[:], z_sb[:])

            with nc.named_scope("p4_allreduce"):
                nc.gpsimd.collective_compute(
                    "AllReduce",
                    mybir.AluOpType.add,
                    replica_groups=[CORE_IDS],
                    ins=[ar_in.opt()],
                    outs=[ar_out.opt()],
                )

            # ===========================================================
            # phase 5: replicated tail (scale, bias, softmax, cls head)
            # ===========================================================
            with tc.tile_pool(name="p5sb", bufs=1) as sb5, tc.tile_pool(
                name="p5ps", bufs=1, space="PSUM"
            ) as ps5, nc.named_scope("p5_tail"):
                z4 = sb5.tile([CLS, B], F32)
                nc.sync.dma_start(z4[:], ar_out[:])
                tot64 = sb5.tile([1, B], F16)
                # tot64 col j = total of batch j = recv[j//8][TOT_OFF + j%8]
                nc.sync.dma_start(
                    tot64[0:1, :], recv[:, TOT_OFF : TOT_OFF + B_LOC]
                )
                ts32 = sb5.tile([1, B], F32)
                nc.vector.tensor_copy(ts32[:], tot64[:])

                ptz = ps5.tile([B, CLS], F32, tag="ptz")
                nc.tensor.transpose(ptz[:], z4[:], identf[0:CLS, 0:CLS])
                ptt = ps5.tile([B, 1], F32, tag="ptt")
                nc.tensor.transpose(ptt[:], ts32[:], identf[0:1, 0:1])

                sq = sb5.tile([B, 1], F32)
                nc.scalar.activation(
                    sq[:],
                    ptt[:],
                    mybir.ActivationFunctionType.Sqrt,
                    bias=normc_col[:],
                    scale=TOT_SCALE,
                )
                sinv = sb5.tile([B, 1], F32)
                nc.vector.reciprocal(sinv[:], sq[:])
                # logits = z*s + kb, written straight into the softmax input
                nc.vector.scalar_tensor_tensor(
                    merged[:, 0:CLS],
                    ptz[:],
                    sinv[:],
                    kb_sb[:],
                    op0=mybir.AluOpType.mult,
                    op1=mybir.AluOpType.add,
                )
                # partition j holds batch j (natural order)
                nc.sync.dma_start(logits_d[:], merged[:, 0:CLS])

                # softmax over 16 features; no max-subtract (|merged| <= ~6)
                esb = sb5.tile([B, 4 * CLS], F32)
                ssum = sb5.tile([B, 1], F32)
                nc.scalar.activation(
                    esb[:],
                    merged[:],
                    mybir.ActivationFunctionType.Exp,
                    accum_out=ssum[:],
                )
                rinv = sb5.tile([B, 1], F32)
                nc.vector.reciprocal(rinv[:], ssum[:])
                nc.vector.tensor_scalar_mul(smx[:, 0 : 4 * CLS], esb[:], rinv[:])

                pmt = ps5.tile([4 * CLS + 1, B], F32, tag="pmt")
                nc.tensor.transpose(pmt[:], smx[:], identf[0:B, 0:B])
                mt = sb5.tile([4 * CLS + 1, B], F32)
                nc.scalar.copy(mt[:], pmt[:])
                pxm = ps5.tile([B, CLS], F32, tag="pxm")
                nc.tensor.matmul(pxm[:], mt[:], wc_sb[:], start=True, stop=True)
                xm_out = sb5.tile([B, CLS], F32)
                nc.scalar.copy(xm_out[:], pxm[:])
                nc.sync.dma_start(xmerge_d[:], xm_out[:])

    return nc


_NC_CACHE = None


def _get_nc():
    global _NC_CACHE
    if _NC_CACHE is None:
        _NC_CACHE = _build_nc()
    return _NC_CACHE


_PREP_CACHE = {}


def _prep_weights(inputs):
    """Host-side weight folding/layout (cached across calls)."""
    key = id(inputs.get("fc0_w"))
    if key in _PREP_CACHE:
        return _PREP_CACHE[key]

    fc0_w = np.asarray(inputs["fc0_w"], dtype=np.float32)
    fc0_b = np.asarray(inputs["fc0_b"], dtype=np.float32)
    fc1_w = np.asarray(inputs["fc1_w"], dtype=np.float32)
    fc1_b = np.asarray(inputs["fc1_b"], dtype=np.float32)
    fc2_w = np.asarray(inputs["fc2_w"], dtype=np.float32)
    fc2_b = np.asarray(inputs["fc2_b"], dtype=np.float32)
    cls_w = np.asarray(inputs["cls_w"], dtype=np.float32)
    cls_b = np.asarray(inputs["cls_b"], dtype=np.float32)

    # symmetry-folded fc0 coefficients
    resh = fc0_w.T.reshape(M, M, O0)  # [m, n, o]
    WA = resh[0:128, :, :].copy()
    WA[:, 128:M, :] += np.transpose(resh[128:M, 0:128, :], (1, 0, 2))
    WBp = np.zeros((8 * RB, 19, O0), dtype=np.float32)
    WBp[0:19] = resh[128:M, 128:M, :]

    w0_cores = []
    for d in range(N_CORES):
        w_slice = np.zeros((SLICE, O0), dtype=np.float32)
        w_slice[0 : RA * M] = WA[RA * d : RA * (d + 1)].reshape(RA * M, O0)
        w_slice[RA * M : RA * M + RB * 19] = WBp[RB * d : RB * (d + 1)].reshape(
            RB * 19, O0
        )
        # device layout [p][k][o], i_local = 128*k + p
        w0_cores.append(
            np.ascontiguousarray(
                w_slice.reshape(KCH, 128, O0)
                .transpose(1, 0, 2)
                .reshape(128, KCH * O0)
                .astype(np.float16)
            )
        )

    w1t = np.ascontiguousarray(
        fc1_w.T.reshape(NB, 128, HID).transpose(1, 0, 2).reshape(128, NB * HID)
    )
    w2t = np.ascontiguousarray(fc2_w.T)
    wct = np.ascontiguousarray(
        np.concatenate([cls_w.T, cls_b.reshape(1, CLS)], axis=0)
    )
    kb = fc2_w @ (fc1_w @ fc0_b + fc1_b) + fc2_b
    kb64 = np.ascontiguousarray(np.broadcast_to(kb, (B, CLS)).copy())

    out = (w0_cores, w1t, w2t, wct, kb64)
    _PREP_CACHE[key] = out
    return out


def _make_in_maps(inputs):
    x1 = np.ascontiguousarray(inputs["x1"], dtype=np.float32).reshape(B, C, L)
    x2 = np.ascontiguousarray(inputs["x2"], dtype=np.float32).reshape(B, C, L)
    x3 = np.ascontiguousarray(inputs["x3"], dtype=np.float32).reshape(B, C, L)
    xc = np.concatenate([x1, x2, x3], axis=1)  # (B, M, L)
    xp = np.zeros((B, M, LCH * 128), dtype=np.float32)
    xp[:, :, 0:L] = xc
    # device layout [b][p][lc*M + m] = x[b, m, 128*lc + p]
    xall = np.ascontiguousarray(
        xp.reshape(B, M, LCH, 128)
        .transpose(0, 3, 2, 1)
        .reshape(B, 128, LCH * M)
        .astype(np.float16)
    )

    w0_cores, w1t, w2t, wct, kb64 = _prep_weights(inputs)

    # batch order is natural end to end
    xm = np.ascontiguousarray(
        np.concatenate(
            [
                np.asarray(inputs["x11"], dtype=np.float32),
                np.asarray(inputs["x21"], dtype=np.float32),
                np.asarray(inputs["x31"], dtype=np.float32),
            ],
            axis=1,
        )
    )

    in_maps = []
    for cidx in range(N_CORES):
        sl = slice(B_LOC * cidx, B_LOC * (cidx + 1))
        in_maps.append(
            {
                "xall": np.ascontiguousarray(xall[sl]),
                "w0f": w0_cores[cidx],
                "w1t": w1t,
                "w2t": w2t,
                "wct": wct,
                "kb": kb64,
                "xm": xm,
            }
        )
    return in_maps


def run(inputs, trace=False, **kwargs):
    nc = _get_nc()
    in_maps = _make_in_maps(inputs)
    res = run_bass_kernel_spmd(nc, in_maps, CORE_IDS, trace=trace, **kwargs)
    out = res.results[0]
    logits = np.asarray(out["logits"], dtype=np.float32)
    x_merge = np.asarray(out["x_merge"], dtype=np.float32)
    return (logits, x_merge), res


def kernel(**inputs):
    (logits, x_merge), _ = run(inputs, trace=False)
    return logits, x_merge


# revision 48
# speedup vs baseline: 1.0706x; 1.0706x over previous
"""Bilinear pooling kernel for 8 Trainium2 NeuronCores (Bass/Tile).

Math (matches the jax reference):
  x = concat([x1, x2, x3], channel) -> (B=64, M=147, L=3136)
  phi_b = x_b @ x_b.T                              (147, 147), symmetric
  phi = sign(phi) * sqrt(|phi| + EPS)              (signed sqrt)
  phi = phi / sqrt(sum(phi^2 + EPS) + 1.0)         (per-batch normalize)
  h = phi_vec @ fc0_w.T + fc0_b                    (64, 1024)
  y = h @ fc1_w.T + fc1_b                          (64, 64)
  logits = y @ fc2_w.T + fc2_b                     (64, 4)
  merged = softmax(concat([logits, x11, x21, x31]))
  x_merge = merged @ cls_w.T + cls_b               (64, 4)

Distribution (v2):
  - phase 1 is batch-parallel (8 batches/core).  Because phi is symmetric,
    only rows 0:128 (A block, 128x147) and the 19x19 diagonal block are
    computed; the mirror block is recovered by folding fc0's weights
    host-side.  The 19x19 block is built from 5 stacked matmuls (5 l-chunks
    side by side in the PE array) instead of 25 thin ones.
  - normalization is deferred: phase 1 ships UNNORMALIZED signed-sqrt phi
    (fp16) plus per-batch |phi| totals; the 1/sqrt(total+C) scale is applied
    to the 4-wide logits after the final AllReduce (everything in between is
    linear in phi).
  - fc0 is contraction(i)-sharded: an AllToAll gives each core a 2432-wide
    i-slice of every batch's phi vector (~150 KB per collective vs 2.8 MB
    for the old AllGather).  Each core computes partial h (1024) -> partial
    y (64) -> partial z = W2 y (4x64); one tiny AllReduce of (4,64)+totals
    finishes the linear chain, then the softmax tail runs replicated.
  - all DRAM inputs are laid out host-side in exact SBUF layout so every
    DMA moves contiguous multi-KB partition lines.
"""

import sys

sys.path.insert(0, "/opt/trn_rl_repo")

import numpy as np

import concourse.bass as bass
import concourse.tile as tile
from concourse import masks, mybir
from concourse.bass_utils import run_bass_kernel_spmd
import bass_rust
from bass_rust import ScopedClock

# ---------------------------------------------------------------------------
# Workaround: this toolchain's walrus accepts only ONE semaphore wait per
# instruction, but Tile can attach several.  Split excess waits onto
# same-engine nops placed immediately before the instruction (same engine
# => executed in order, so synchronization semantics are unchanged).
# ---------------------------------------------------------------------------
_MAX_WAITS = 1
_ws_counter = [0]


def _split_excess_waits(obb):
    for bb, insts in list(obb.items()):
        new_list = []
        for inst in insts:
            info = inst.sync_info
            if info is not None and len(info.on_wait) > _MAX_WAITS:
                waits = list(info.on_wait)
                excess = waits[:-_MAX_WAITS]
                keep = waits[-_MAX_WAITS:]
                for i in range(0, len(excess), _MAX_WAITS):
                    _ws_counter[0] += 1
                    nop = mybir.InstNoOp(
                        name=f"WS-{_ws_counter[0]}",
                        sync_info=bass_rust.SyncInfo(
                            on_wait=excess[i : i + _MAX_WAITS], on_update=[]
                        ),
                        bass_nofuse=True,
                        engine=inst.engine,
                    )
                    new_list.append(nop)
                inst.sync_info = bass_rust.SyncInfo(
                    on_wait=keep, on_update=list(info.on_update)
                )
            new_list.append(inst)
        obb[bb] = new_list


_RealTCW = tile.TileClockWait


class _TCWWrapper:
    def __init__(self, *args, **kwargs):
        self._inner = _RealTCW(*args, **kwargs)
        self._obb = (
            args[1] if len(args) > 1 else kwargs["ordered_instructions_by_block"]
        )

    def __getattr__(self, name):
        return getattr(self._inner, name)

    def assign_waits(self, bb_name):
        self._inner.assign_waits(bb_name)
        _split_excess_waits(self._obb)


tile.TileClockWait = _TCWWrapper


def _split_drain_and_barrier(self, tick_clock, wait_clock):
    nc = self.nc
    drain_inst = nc.sync.drain()
    wait_clock.add_sem_waits(
        drain_inst.ins, ScopedClock({None: tick_clock.global_clock})
    )
    info = drain_inst.ins.sync_info
    if info is not None and len(info.on_wait) > _MAX_WAITS:
        waits = list(info.on_wait)
        drain_inst.ins.sync_info = bass_rust.SyncInfo(
            on_wait=waits[:_MAX_WAITS], on_update=list(info.on_update)
        )
        rest = waits[_MAX_WAITS:]
        while rest:
            chunk, rest = rest[:_MAX_WAITS], rest[_MAX_WAITS:]
            nop_inst = nc.sync.nop(nofuse=True, hint="tail_drain_split")
            nop_inst.ins.sync_info = bass_rust.SyncInfo(on_wait=chunk, on_update=[])
    nc.all_engine_barrier()
    assert self.sems is not None
    popped = nc._tile_sem_poison_stack.pop()
    assert popped is self._sem_poison
    nc.clear_and_free_semaphores(list(self.sems.allocated().values()))
    nc.all_engine_barrier()


tile.TileContext._drain_and_barrier = _split_drain_and_barrier

# ---------------------------------------------------------------------------
# Problem constants (hardcoded per the spec)
# ---------------------------------------------------------------------------
N_CORES = 8
CORE_IDS = list(range(N_CORES))
B = 64
B_LOC = B // N_CORES  # 8 batches per core
C = 49
L = 3136  # 56*56
LCH = 25  # l-chunks of 128 (last one zero-padded: 3136 = 24*128 + 64)
M = 147  # 3*49 channels
O0 = 1024  # fc0 out features
HID = 64
CLS = 4
EPS = 1e-8
MM = M * M
# normalizer: sum(phi_ss^2 + EPS) + 1.0 == sum|phi| + 2*MM*EPS + 1.0
NORM_C = float(2 * MM * EPS + 1.0)
TOT_SCALE = 256.0  # |phi| totals are carried as fp16 scaled by 1/256

RA = 16  # A-block rows per destination core
RB = 3  # padded-B rows per destination core (B 19x19 padded to 24 rows)
SLICE = 2432  # = 19*128 per-batch per-dest phi slice (2352 A + 57 B + 23 pad)
KCH = SLICE // 128  # 19 fc0 contraction chunks
NB = 8  # fc0 output blocks of 128

F32 = mybir.dt.float32
F16 = mybir.dt.float16


def _build_nc():
    nc = bass.Bass()

    # -- external I/O ------------------------------------------------------
    # all big tensors arrive in exact SBUF layout (partition-major).
    xall_d = nc.dram_tensor("xall", [B_LOC, 128, LCH * M], F16, kind="ExternalInput")
    w0_d = nc.dram_tensor("w0f", [128, KCH * O0], F16, kind="ExternalInput")
    w1_d = nc.dram_tensor("w1t", [128, NB * HID], F32, kind="ExternalInput")
    w2_d = nc.dram_tensor("w2t", [HID, CLS], F32, kind="ExternalInput")
    wct_d = nc.dram_tensor("wct", [4 * CLS + 1, CLS], F32, kind="ExternalInput")
    kb_d = nc.dram_tensor("kb", [B, CLS], F32, kind="ExternalInput")
    xm_d = nc.dram_tensor("xm", [B, 3 * CLS], F32, kind="ExternalInput")
    logits_d = nc.dram_tensor("logits", [B, CLS], F32, kind="ExternalOutput")
    xmerge_d = nc.dram_tensor("x_merge", [B, CLS], F32, kind="ExternalOutput")

    IN_H = B_LOC * SLICE  # 19456 elems per dest row
    # the 8 per-batch |phi| totals ride inside the 23-elem pad of the last
    # batch slice (W rows there are zero, so fc0 ignores them)
    TOT_OFF = 7 * SLICE + RA * M + RB * 19

    with tile.TileContext(nc) as tc:
        with tc.tile_pool(name="dram", bufs=1, space="DRAM") as dram, tc.tile_pool(
            name="const", bufs=1
        ) as const:
            a2a_in = dram.tile([N_CORES, IN_H], F16)
            recv = dram.tile([N_CORES, IN_H], F16)
            ar_in = dram.tile([CLS, B], F32)
            ar_out = dram.tile([CLS, B], F32, addr_space="Shared")

            # -- constants ----------------------------------------------
            identf = const.tile([128, 128], F32)
            masks.make_identity(nc, identf[:])
            ones16 = const.tile([128, 8], F16)
            nc.gpsimd.memset(ones16[:], 1.0)
            eps_col = const.tile([128, 1], F32)
            nc.gpsimd.memset(eps_col[:], EPS)
            normc_col = const.tile([B, 1], F32)
            nc.gpsimd.memset(normc_col[:], NORM_C)
            zpad = const.tile([64, SLICE - RA * M - RB * 19], F16)
            nc.gpsimd.memset(zpad[:], 0.0)

            # ===========================================================
            # phase 0: input loads.  x batches first (they gate phase 1),
            # then the fc0 weight slab (only needed much later).  Split
            # across BOTH hwdge engines: descriptor generation and ring
            # drain interlock per engine, so two pipelines double the
            # effective issue rate.  Small/const loads go to gpsimd.
            # ===========================================================
            xt = const.tile([128, B_LOC, LCH, M], F16)
            for b in range(B_LOC):
                nc.sync.dma_start(
                    xt[:, b], xall_d[b].rearrange("p (lc m) -> p lc m", lc=LCH)
                )
            w_sb = const.tile([128, KCH, O0], F16)
            for kg in range(4):
                k0, k1 = 5 * kg, min(5 * (kg + 1), KCH)
                nc.sync.dma_start(
                    w_sb[:, k0:k1],
                    w0_d[:, k0 * O0 : k1 * O0].rearrange("p (k o) -> p k o", o=O0),
                )
            w1_sb = const.tile([128, NB, HID], F32)
            nc.scalar.dma_start(
                w1_sb[:], w1_d.rearrange("p (n h) -> p n h", h=HID)
            )
            w2_sb = const.tile([HID, CLS], F32)
            nc.scalar.dma_start(w2_sb[:], w2_d[:])
            wc_sb = const.tile([4 * CLS + 1, CLS], F32)
            nc.scalar.dma_start(wc_sb[:], wct_d[:])
            kb_sb = const.tile([B, CLS], F32)
            nc.scalar.dma_start(kb_sb[:], kb_d[:])
            xm_sb = const.tile([B, 3 * CLS], F32)
            nc.scalar.dma_start(xm_sb[:], xm_d[:])
            merged = const.tile([B, 4 * CLS], F32)
            nc.vector.tensor_copy(merged[:, CLS:], xm_sb[:])
            smx = const.tile([B, 4 * CLS + 1], F32)
            nc.vector.memset(smx[:, 4 * CLS :], 1.0)
            # zero the 23-elem pad of every (dest, batch) slice once
            nc.scalar.dma_start(
                a2a_in.rearrange("d (b i) -> (d b) i", b=B_LOC)[
                    :, RA * M + RB * 19 :
                ],
                zpad[:],
            )
            # phiT lives here so the first transpose can issue mid-phase-1
            phiT = const.tile([128, KCH, B], F16)

            # per-batch |phi| row sums, accumulated across phase 1
            rsum = const.tile([128, B_LOC, 2], F32)
            rsumB = const.tile([M - 128, B_LOC], F32)
            # padded signed-sqrt B block; rows 19:24 zeroed once via DMA
            # (engine APs must start at a 32-aligned partition)
            nBp = const.tile([8 * RB, 19], F16)
            nc.gpsimd.dma_start(nBp[19 : 8 * RB, :], zpad[0 : 8 * RB - 19, 0:19])
            # B columns repacked at 32-col pitch so the per-chunk diagonal
            # blocks land on 32-aligned partitions; cols 19:32 stay zero
            bpack = const.tile([128, LCH, 32], F16)
            nc.vector.memset(bpack[:], 0.0)

            # ===========================================================
            # phase 1: bilinear + signed sqrt, per batch (normalization
            # deferred to the tail).  PE stream has NO mid-stream stalls.
            # ===========================================================
            with tc.tile_pool(name="p1sb", bufs=3) as sb, tc.tile_pool(
                name="p1psA", bufs=3, space="PSUM"
            ) as psA, tc.tile_pool(
                name="p1psB", bufs=2, space="PSUM"
            ) as psB, nc.named_scope("p1_bilinear"):
                for b in range(B_LOC):
                    bl = b

                    # A block: phi rows 0:128 x cols 0:147
                    pA = psA.tile([128, M], F32, tag="pA")
                    for lc in range(LCH):
                        nc.tensor.matmul(
                            pA[:],
                            xt[:, b, lc, 0:128],
                            xt[:, b, lc, :],
                            start=(lc == 0),
                            stop=(lc == LCH - 1),
                        )
                    # B diagonal 19x19: 4 l-chunks stacked side by side at
                    # 32-col pitch; diag 32-blocks of the 128x128 result
                    # hold the per-chunk contributions, summed below on DVE.
                    nc.vector.tensor_copy(bpack[:, :, 0:19], xt[:, b, :, 128:M])
                    pB5 = psB.tile([128, 128], F32, tag="pB5")
                    for s in range(6):
                        ap = bpack[:, 4 * s : 4 * s + 4, :]
                        nc.tensor.matmul(
                            pB5[:], ap, ap, start=(s == 0), stop=False
                        )
                    ap = bpack[:, 24, :]
                    nc.tensor.matmul(
                        pB5[0:32, 0:32], ap, ap, start=False, stop=True
                    )

                    # norm chain (ACT + DVE only; PE keeps streaming)
                    sgnA = sb.tile([128, M], F32, tag="sgnA")
                    absA = sb.tile([128, M], F32, tag="absA")
                    nc.scalar.activation(
                        sgnA[:], pA[:], mybir.ActivationFunctionType.Sign
                    )
                    nc.scalar.activation(
                        absA[:], pA[:], mybir.ActivationFunctionType.Abs
                    )
                    nc.vector.reduce_sum(
                        rsum[:, b, 0:1], absA[:], axis=mybir.AxisListType.X
                    )
                    # mirror columns 128:147 count twice in the full |phi| sum
                    nc.vector.reduce_sum(
                        rsum[:, b, 1:2], absA[:, 128:M], axis=mybir.AxisListType.X
                    )
                    sqA = sb.tile([128, M], F32, tag="sqA")
                    nc.scalar.activation(
                        sqA[:],
                        absA[:],
                        mybir.ActivationFunctionType.Sqrt,
                        bias=eps_col[:],
                    )
                    nA = sb.tile([128, M], F16, tag="nA")
                    nc.vector.tensor_mul(nA[:], sqA[:], sgnA[:])
                    # early phi writes ride gpsimd (its queue is free; the
                    # collective instruction would block anything behind it);
                    # later ones go to scalar, whose small loads have drained
                    wr_eng = nc.gpsimd if b < 4 else nc.scalar
                    wr_eng.dma_start(
                        a2a_in[:, bl * SLICE : bl * SLICE + RA * M].rearrange(
                            "d (r m) -> d r m", r=RA
                        ),
                        nA[:],
                    )

                    bsum = sb.tile([19, 19], F32, tag="bsum")
                    nc.vector.tensor_copy(bsum[:], pB5[0:19, 0:19])
                    nc.vector.tensor_add(bsum[:], bsum[:], pB5[32:51, 32:51])
                    nc.vector.tensor_add(bsum[:], bsum[:], pB5[64:83, 64:83])
                    nc.vector.tensor_add(bsum[:], bsum[:], pB5[96:115, 96:115])
                    sgnB = sb.tile([19, 19], F32, tag="sgnB")
                    absB = sb.tile([19, 19], F32, tag="absB")
                    nc.scalar.activation(
                        sgnB[:], bsum[:], mybir.ActivationFunctionType.Sign
                    )
                    nc.scalar.activation(
                        absB[:], bsum[:], mybir.ActivationFunctionType.Abs
                    )
                    nc.vector.reduce_sum(
                        rsumB[:, b : b + 1], absB[:], axis=mybir.AxisListType.X
                    )
                    sqB = sb.tile([19, 19], F32, tag="sqB")
                    nc.scalar.activation(
                        sqB[:],
                        absB[:],
                        mybir.ActivationFunctionType.Sqrt,
                        bias=eps_col[0:19],
                    )
                    nc.vector.tensor_mul(nBp[0:19, :], sqB[:], sgnB[:])
                    wr_eng.dma_start(
                        a2a_in[
                            :, bl * SLICE + RA * M : bl * SLICE + RA * M + RB * 19
                        ].rearrange("d (s c) -> d s c", s=RB),
                        nBp[:],
                    )

                # per-batch |phi| totals: cross-partition sum via one
                # fp16 ones-matmul (PE is idle by now), scaled by 1/256
                # to fit fp16 on the wire.
                rs_sum = sb.tile([128, B_LOC], F32, tag="rs_sum")
                nc.vector.tensor_add(rs_sum[:], rsum[:, :, 0], rsum[:, :, 1])
                rs16 = sb.tile([128, B_LOC], F16, tag="rs16")
                nc.vector.tensor_scalar_mul(rs16[:], rs_sum[:], 1.0 / TOT_SCALE)
                rsB16 = sb.tile([M - 128, B_LOC], F16, tag="rsB16")
                nc.vector.tensor_scalar_mul(rsB16[:], rsumB[:], 1.0 / TOT_SCALE)
                tot_ps = psB.tile([8, 8], F32, tag="tot")
                nc.tensor.matmul(
                    tot_ps[:], ones16[:, :], rs16[:], start=True, stop=False
                )
                nc.tensor.matmul(
                    tot_ps[:],
                    ones16[0 : M - 128, :],
                    rsB16[:],
                    start=False,
                    stop=True,
                )
                tot16 = sb.tile([8, 8], F16, tag="tot16")
                nc.scalar.copy(tot16[:], tot_ps[:])
                nc.scalar.dma_start(
                    a2a_in[:, TOT_OFF : TOT_OFF + B_LOC], tot16[:]
                )

            with nc.named_scope("p2_a2a"):
                nc.gpsimd.collective_compute(
                    "AllToAll",
                    mybir.AluOpType.bypass,
                    replica_groups=[CORE_IDS],
                    ins=[a2a_in.opt()],
                    outs=[recv.opt()],
                )

            # ===========================================================
            # phase 3: transpose received phi slices, fc0/fc1/fc2 partials
            # column j of phiT holds batch j (natural order)
            # ===========================================================
            with tc.tile_pool(name="p3sb", bufs=1) as sb3, tc.tile_pool(
                name="p3ps", bufs=1, space="PSUM"
            ) as ps3, nc.named_scope("p3_fc0"):
                nc.sync.dma_start_transpose(
                    phiT[:],
                    recv.rearrange("d (b i) -> (d b) i", b=B_LOC),
                )

                h_sb = sb3.tile([128, NB, B], F32)
                for ob in range(NB):
                    ph = ps3.tile([128, B], F32, tag=f"h{ob % 2}", bufs=2)
                    for k in range(KCH):
                        nc.tensor.matmul(
                            ph[:],
                            w_sb[:, k, 128 * ob : 128 * (ob + 1)],
                            phiT[:, k, :],
                            start=(k == 0),
                            stop=(k == KCH - 1),
                        )
                    nc.scalar.copy(h_sb[:, ob, :], ph[:])

                py = ps3.tile([HID, B], F32, tag="py")
                for ob in range(NB):
                    nc.tensor.matmul(
                        py[:],
                        w1_sb[:, ob, :],
                        h_sb[:, ob, :],
                        start=(ob == 0),
                        stop=(ob == NB - 1),
                    )
                y_sb = sb3.tile([HID, B], F32)
                nc.vector.tensor_copy(y_sb[:], py[:])
                pz = ps3.tile([CLS, B], F32, tag="pz")
                nc.tensor.matmul(pz[:], w2_sb[:], y_sb[:], start=True, stop=True)
                z_sb = sb3.tile([CLS, B], F32)
                nc.scalar.copy(z_sb[:], pz[:])
                nc.scalar.dma_start(ar_in[:], z_sb[:])

            with nc.named_scope("p4_allreduce"):
                nc.gpsimd.collective_compute(
                    "AllReduce",
                    mybir.AluOpType.add,
                    replica_groups=[CORE_IDS],
                    ins=[ar_in.opt()],
                    outs=[ar_out.opt()],
                )

            # ===========================================================
            # phase 5: replicated tail (scale, bias, softmax, cls head)
            # ===========================================================
            with tc.tile_pool(name="p5sb", bufs=1) as sb5, tc.tile_pool(
                name="p5ps", bufs=1, space="PSUM"
            ) as ps5, nc.named_scope("p5_tail"):
                z4 = sb5.tile([CLS, B], F32)
                nc.sync.dma_start(z4[:], ar_out[:])
                tot64 = sb5.tile([1, B], F16)
                # tot64 col j = total of batch j = recv[j//8][TOT_OFF + j%8]
                nc.sync.dma_start(
                    tot64[0:1, :], recv[:, TOT_OFF : TOT_OFF + B_LOC]
                )
                ts32 = sb5.tile([1, B], F32)
                nc.vector.tensor_copy(ts32[:], tot64[:])

                ptz = ps5.tile([B, CLS], F32, tag="ptz")
                nc.tensor.transpose(ptz[:], z4[:], identf[0:CLS, 0:CLS])
                ptt = ps5.tile([B, 1], F32, tag="ptt")
                nc.tensor.transpose(ptt[:], ts32[:], identf[0:1, 0:1])

                sq = sb5.tile([B, 1], F32)
                nc.scalar.activation(
                    sq[:],
                    ptt[:],
                    mybir.ActivationFunctionType.Sqrt,
                    bias=normc_col[:],
                    scale=TOT_SCALE,
                )
                sinv = sb5.tile([B, 1], F32)
                nc.vector.reciprocal(sinv[:], sq[:])
                # logits = z*s + kb, written straight into the softmax input
                nc.vector.scalar_tensor_tensor(
                    merged[:, 0:CLS],
                    ptz[:],
                    sinv[:],
                    kb_sb[:],
                    op0=mybir.AluOpType.mult,
                    op1=mybir.AluOpType.add,
                )
                # partition j holds batch j (natural order)
                nc.sync.dma_start(logits_d[:], merged[:, 0:CLS])

                # softmax over 16 features; no max-subtract (|merged| <= ~6)
                esb = sb5.tile([B, 4 * CLS], F32)
                ssum = sb5.tile([B, 1], F32)
                nc.scalar.activation(
                    esb[:],
                    merged[:],
                    mybir.ActivationFunctionType.Exp,
                    accum_out=ssum[:],
                )
                rinv = sb5.tile([B, 1], F32)
                nc.vector.reciprocal(rinv[:], ssum[:])
                nc.vector.tensor_scalar_mul(smx[:, 0 : 4 * CLS], esb[:], rinv[:])

                pmt = ps5.tile([4 * CLS + 1, B], F32, tag="pmt")
                nc.tensor.transpose(pmt[:], smx[:], identf[0:B, 0:B])
                mt = sb5.tile([4 * CLS + 1, B], F32)
                nc.scalar.copy(mt[:], pmt[:])
                pxm = ps5.tile([B, CLS], F32, tag="pxm")
                nc.tensor.matmul(pxm[:], mt[:], wc_sb[:], start=True, stop=True)
                xm_out = sb5.tile([B, CLS], F32)
                nc.scalar.copy(xm_out[:], pxm[:])
                nc.sync.dma_start(xmerge_d[:], xm_out[:])

    return nc


_NC_CACHE = None


def _get_nc():
    global _NC_CACHE
    if _NC_CACHE is None:
        _NC_CACHE = _build_nc()
    return _NC_CACHE


_PREP_CACHE = {}


def _prep_weights(inputs):
    """Host-side weight folding/layout (cached across calls)."""
    key = id(inputs.get("fc0_w"))
    if key in _PREP_CACHE:
        return _PREP_CACHE[key]

    fc0_w = np.asarray(inputs["fc0_w"], dtype=np.float32)
    fc0_b = np.asarray(inputs["fc0_b"], dtype=np.float32)
    fc1_w = np.asarray(inputs["fc1_w"], dtype=np.float32)
    fc1_b = np.asarray(inputs["fc1_b"], dtype=np.float32)
    fc2_w = np.asarray(inputs["fc2_w"], dtype=np.float32)
    fc2_b = np.asarray(inputs["fc2_b"], dtype=np.float32)
    cls_w = np.asarray(inputs["cls_w"], dtype=np.float32)
    cls_b = np.asarray(inputs["cls_b"], dtype=np.float32)

    # symmetry-folded fc0 coefficients
    resh = fc0_w.T.reshape(M, M, O0)  # [m, n, o]
    WA = resh[0:128, :, :].copy()
    WA[:, 128:M, :] += np.transpose(resh[128:M, 0:128, :], (1, 0, 2))
    WBp = np.zeros((8 * RB, 19, O0), dtype=np.float32)
    WBp[0:19] = resh[128:M, 128:M, :]

    w0_cores = []
    for d in range(N_CORES):
        w_slice = np.zeros((SLICE, O0), dtype=np.float32)
        w_slice[0 : RA * M] = WA[RA * d : RA * (d + 1)].reshape(RA * M, O0)
        w_slice[RA * M : RA * M + RB * 19] = WBp[RB * d : RB * (d + 1)].reshape(
            RB * 19, O0
        )
        # device layout [p][k][o], i_local = 128*k + p
        w0_cores.append(
            np.ascontiguousarray(
                w_slice.reshape(KCH, 128, O0)
                .transpose(1, 0, 2)
                .reshape(128, KCH * O0)
                .astype(np.float16)
            )
        )

    w1t = np.ascontiguousarray(
        fc1_w.T.reshape(NB, 128, HID).transpose(1, 0, 2).reshape(128, NB * HID)
    )
    w2t = np.ascontiguousarray(fc2_w.T)
    wct = np.ascontiguousarray(
        np.concatenate([cls_w.T, cls_b.reshape(1, CLS)], axis=0)
    )
    kb = fc2_w @ (fc1_w @ fc0_b + fc1_b) + fc2_b
    kb64 = np.ascontiguousarray(np.broadcast_to(kb, (B, CLS)).copy())

    out = (w0_cores, w1t, w2t, wct, kb64)
    _PREP_CACHE[key] = out
    return out


def _make_in_maps(inputs):
    x1 = np.ascontiguousarray(inputs["x1"], dtype=np.float32).reshape(B, C, L)
    x2 = np.ascontiguousarray(inputs["x2"], dtype=np.float32).reshape(B, C, L)
    x3 = np.ascontiguousarray(inputs["x3"], dtype=np.float32).reshape(B, C, L)
    xc = np.concatenate([x1, x2, x3], axis=1)  # (B, M, L)
    xp = np.zeros((B, M, LCH * 128), dtype=np.float32)
    xp[:, :, 0:L] = xc
    # device layout [b][p][lc*M + m] = x[b, m, 128*lc + p]
    xall = np.ascontiguousarray(
        xp.reshape(B, M, LCH, 128)
        .transpose(0, 3, 2, 1)
        .reshape(B, 128, LCH * M)
        .astype(np.float16)
    )

    w0_cores, w1t, w2t, wct, kb64 = _prep_weights(inputs)

    # batch order is natural end to end
    xm = np.ascontiguousarray(
        np.concatenate(
            [
                np.asarray(inputs["x11"], dtype=np.float32),
                np.asarray(inputs["x21"], dtype=np.float32),
                np.asarray(inputs["x31"], dtype=np.float32),
            ],
            axis=1,
        )
    )

    in_maps = []
    for cidx in range(N_CORES):
        sl = slice(B_LOC * cidx, B_LOC * (cidx + 1))
        in_maps.append(
            {
                "xall": np.ascontiguousarray(xall[sl]),
                "w0f": w0_cores[cidx],
                "w1t": w1t,
                "w2t": w2t,
                "wct": wct,
                "kb": kb64,
                "xm": xm,
            }
        )
    return in_maps


def run(inputs, trace=False, **kwargs):
    nc = _get_nc()
    in_maps = _make_in_maps(inputs)
    res = run_bass_kernel_spmd(nc, in_maps, CORE_IDS, trace=trace, **kwargs)
    out = res.results[0]
    logits = np.asarray(out["logits"], dtype=np.float32)
    x_merge = np.asarray(out["x_merge"], dtype=np.float32)
    return (logits, x_merge), res


def kernel(**inputs):
    (logits, x_merge), _ = run(inputs, trace=False)
    return logits, x_merge
